# revision 5
# baseline (speedup 1.0000x reference)
"""Trainium2 Bass kernel for nn_RadialPredictionLayer (retrieval_knn).

Computes out[n, c] = -sqrt(max(||x_n||^2 + ||p_c||^2 - 2 * x_n . p_c, 0))
for x [32768, 1024] fp32 and prototypes [1024, 1024] fp32.

The layer's prototypes are a fixed (non-trainable) identity matrix, so the
device kernel specializes on that constant (verified at runtime):
    cross = x @ I^T = x,  ||p_c||^2 = 1
    out[n, c] = -sqrt(1 + ||x_n||^2 - 2 * x[n, c])
which is a pure memory-bound elementwise + row-reduction kernel (no GEMM).
Sharding: data-parallel on the batch axis across 8 NeuronCores; each core
processes a [4096, 1024] row block. If prototypes is ever not the identity,
a host-side exact fallback implements the general formula.
"""

import numpy as np

N_CORES = 8
N_ROWS = 32768
D = 1024
ROWS_PER_CORE = N_ROWS // N_CORES  # 4096
T = 4  # rows per partition per super-tile
SUP = ROWS_PER_CORE // (128 * T)  # super-tiles per core

_cache = {}


def _build_program(rows=ROWS_PER_CORE, debug=False):
    import concourse.bacc as bacc
    import concourse.mybir as mybir
    import concourse.tile as tile

    f32 = mybir.dt.float32
    nc = bacc.Bacc("TRN2", target_bir_lowering=False, debug=debug)
    x = nc.dram_tensor("x", [rows, D], f32, kind="ExternalInput").ap()
    out = nc.dram_tensor("out", [rows, D], f32, kind="ExternalOutput").ap()

    xv = x.rearrange("(s p t) d -> s p (t d)", p=128, t=T)
    ov = out.rearrange("(s p t) d -> s p (t d)", p=128, t=T)

    with tile.TileContext(nc) as tc:
        with (
            tc.tile_pool(name="xt", bufs=6) as xpool,
            tc.tile_pool(name="sc", bufs=3) as scpool,
            tc.tile_pool(name="b", bufs=6) as bpool,
        ):
            for s in range(rows // (128 * T)):
                xt = xpool.tile([128, T * D], f32)
                nc.sync.dma_start(out=xt[:], in_=xv[s])
                b = bpool.tile([128, T], f32)
                sq = scpool.tile([128, D], f32)
                for t in range(T):
                    blk = xt[:, t * D : (t + 1) * D]
                    bcol = b[:, t : t + 1]
                    # sq = x*x (discarded); b[:, t] = sum(x*x) per row
                    nc.vector.scalar_tensor_tensor(
                        out=sq[:],
                        in0=blk,
                        scalar=1.0,
                        in1=blk,
                        op0=mybir.AluOpType.mult,
                        op1=mybir.AluOpType.mult,
                        accum_out=bcol,
                    )
                    # b = 1 + ||x_row||^2
                    nc.vector.tensor_scalar_add(out=bcol, in0=bcol, scalar1=1.0)
                    # blk = sqrt(-2*x + (1 + ||x_row||^2))   (in place)
                    nc.scalar.activation(
                        out=blk,
                        in_=blk,
                        func=mybir.ActivationFunctionType.Sqrt,
                        bias=bcol,
                        scale=-2.0,
                    )
                # negate the whole super-tile on the otherwise-idle gpsimd
                nc.gpsimd.tensor_scalar_mul(out=xt[:], in0=xt[:], scalar1=-1.0)
                nc.sync.dma_start(out=ov[s], in_=xt[:])
    nc.finalize()
    return nc


def _run_device(x: np.ndarray, trace: bool = False):
    from concourse import bass_utils

    if "nc" not in _cache:
        _cache["nc"] = _build_program()
    nc = _cache["nc"]
    shards = [
        np.ascontiguousarray(x[i * ROWS_PER_CORE : (i + 1) * ROWS_PER_CORE])
        for i in range(N_CORES)
    ]
    res = bass_utils.run_bass_kernel_spmd(
        nc,
        [{"x": s} for s in shards],
        core_ids=list(range(N_CORES)),
        trace=trace,
    )
    out = np.concatenate([r["out"] for r in res.results], axis=0)
    return out, res


def _fallback(x: np.ndarray, prototypes: np.ndarray) -> np.ndarray:
    x = x.astype(np.float32, copy=False)
    p = prototypes.astype(np.float32, copy=False)
    x_sq = np.sum(x * x, axis=1, keepdims=True)
    p_sq = np.sum(p * p, axis=1)
    cross = x @ p.T
    d2 = np.maximum(x_sq + p_sq[None, :] - 2.0 * cross, 0.0)
    return (-np.sqrt(d2)).astype(np.float32)


def _is_identity(p: np.ndarray) -> bool:
    if p.shape != (D, D):
        return False
    if "eye" not in _cache:
        _cache["eye"] = np.eye(D, dtype=np.float32)
    return np.array_equal(np.asarray(p, dtype=np.float32), _cache["eye"])


def kernel(x: np.ndarray, prototypes: np.ndarray) -> np.ndarray:
    x = np.asarray(x)
    prototypes = np.asarray(prototypes)
    if (
        x.shape == (N_ROWS, D)
        and x.dtype == np.float32
        and _is_identity(prototypes)
    ):
        out, _ = _run_device(x)
        return out
    return _fallback(x, prototypes)


# revision 6
# speedup vs baseline: 4.7155x; 4.7155x over previous
"""Trainium2 Bass kernel for nn_RadialPredictionLayer (retrieval_knn).

Computes out[n, c] = -sqrt(max(||x_n||^2 + ||p_c||^2 - 2 * x_n . p_c, 0))
for x [32768, 1024] fp32 and prototypes [1024, 1024] fp32.

The layer's prototypes are a fixed (non-trainable) identity matrix, so the
device kernel specializes on that constant (verified at runtime):
    cross = x @ I^T = x,  ||p_c||^2 = 1
    out[n, c] = -sqrt(1 + ||x_n||^2 - 2 * x[n, c])
which is a pure memory-bound elementwise + row-reduction kernel (no GEMM).
Sharding: data-parallel on the batch axis across 8 NeuronCores; each core
processes a [4096, 1024] row block. If prototypes is ever not the identity,
a host-side exact fallback implements the general formula.
"""

import numpy as np

N_CORES = 8
N_ROWS = 32768
D = 1024
ROWS_PER_CORE = N_ROWS // N_CORES  # 4096
T = 4  # rows per partition per super-tile
SUP = ROWS_PER_CORE // (128 * T)  # super-tiles per core

_cache = {}


def _build_program(rows=ROWS_PER_CORE, debug=False):
    import concourse.bacc as bacc
    import concourse.mybir as mybir
    import concourse.tile as tile

    f32 = mybir.dt.float32
    nc = bacc.Bacc("TRN2", target_bir_lowering=False, debug=debug)
    x = nc.dram_tensor("x", [rows, D], f32, kind="ExternalInput").ap()
    out = nc.dram_tensor("out", [rows, D], f32, kind="ExternalOutput").ap()

    xv = x.rearrange("(s p t) d -> s p (t d)", p=128, t=T)
    ov = out.rearrange("(s p t) d -> s p (t d)", p=128, t=T)

    with tile.TileContext(nc) as tc:
        with (
            tc.tile_pool(name="xt", bufs=6) as xpool,
            tc.tile_pool(name="sc", bufs=3) as scpool,
            tc.tile_pool(name="b", bufs=6) as bpool,
        ):
            for s in range(rows // (128 * T)):
                xt = xpool.tile([128, T * D], f32)
                nc.sync.dma_start(out=xt[:], in_=xv[s])
                b = bpool.tile([128, T], f32)
                sq = scpool.tile([128, D], f32)
                for t in range(T):
                    blk = xt[:, t * D : (t + 1) * D]
                    bcol = b[:, t : t + 1]
                    # sq = x*x (discarded); b[:, t] = sum(x*x) per row
                    nc.vector.scalar_tensor_tensor(
                        out=sq[:],
                        in0=blk,
                        scalar=1.0,
                        in1=blk,
                        op0=mybir.AluOpType.mult,
                        op1=mybir.AluOpType.mult,
                        accum_out=bcol,
                    )
                    # b = 1 + ||x_row||^2
                    nc.vector.tensor_scalar_add(out=bcol, in0=bcol, scalar1=1.0)
                    # blk = sqrt(-2*x + (1 + ||x_row||^2))   (in place)
                    nc.scalar.activation(
                        out=blk,
                        in_=blk,
                        func=mybir.ActivationFunctionType.Sqrt,
                        bias=bcol,
                        scale=-2.0,
                    )
                # negate the whole super-tile in one op
                nc.vector.tensor_scalar_mul(out=xt[:], in0=xt[:], scalar1=-1.0)
                nc.sync.dma_start(out=ov[s], in_=xt[:])
    nc.finalize()
    return nc


def _run_device(x: np.ndarray, trace: bool = False):
    from concourse import bass_utils

    if "nc" not in _cache:
        _cache["nc"] = _build_program()
    nc = _cache["nc"]
    shards = [
        np.ascontiguousarray(x[i * ROWS_PER_CORE : (i + 1) * ROWS_PER_CORE])
        for i in range(N_CORES)
    ]
    res = bass_utils.run_bass_kernel_spmd(
        nc,
        [{"x": s} for s in shards],
        core_ids=list(range(N_CORES)),
        trace=trace,
    )
    out = np.concatenate([r["out"] for r in res.results], axis=0)
    return out, res


def _fallback(x: np.ndarray, prototypes: np.ndarray) -> np.ndarray:
    x = x.astype(np.float32, copy=False)
    p = prototypes.astype(np.float32, copy=False)
    x_sq = np.sum(x * x, axis=1, keepdims=True)
    p_sq = np.sum(p * p, axis=1)
    cross = x @ p.T
    d2 = np.maximum(x_sq + p_sq[None, :] - 2.0 * cross, 0.0)
    return (-np.sqrt(d2)).astype(np.float32)


def _is_identity(p: np.ndarray) -> bool:
    if p.shape != (D, D):
        return False
    if "eye" not in _cache:
        _cache["eye"] = np.eye(D, dtype=np.float32)
    return np.array_equal(np.asarray(p, dtype=np.float32), _cache["eye"])


def kernel(x: np.ndarray, prototypes: np.ndarray) -> np.ndarray:
    x = np.asarray(x)
    prototypes = np.asarray(prototypes)
    if (
        x.shape == (N_ROWS, D)
        and x.dtype == np.float32
        and _is_identity(prototypes)
    ):
        out, _ = _run_device(x)
        return out
    return _fallback(x, prototypes)


# revision 22
# speedup vs baseline: 6.7905x; 1.4400x over previous
"""Trainium2 Bass kernel for nn_RadialPredictionLayer (retrieval_knn).

Computes out[n, c] = -sqrt(max(||x_n||^2 + ||p_c||^2 - 2 * x_n . p_c, 0))
for x [32768, 1024] fp32 and prototypes [1024, 1024] fp32.

The layer's prototypes are a fixed (non-trainable) identity matrix, so the
device kernel specializes on that constant (verified at runtime):
    cross = x @ I^T = x,  ||p_c||^2 = 1
    out[n, c] = -sqrt(1 + ||x_n||^2 - 2 * x[n, c])
which is a pure memory-bound elementwise + row-reduction kernel (no GEMM).
Sharding: data-parallel on the batch axis across 8 NeuronCores; each core
processes a [4096, 1024] row block. If prototypes is ever not the identity,
a host-side exact fallback implements the general formula.
"""

import numpy as np

N_CORES = 8
N_ROWS = 32768
D = 1024
ROWS_PER_CORE = N_ROWS // N_CORES  # 4096
T = 4  # rows per partition per super-tile
SUP = ROWS_PER_CORE // (128 * T)  # super-tiles per core

_cache = {}


def _tile_plan(rows_per_part, tt):
    """Taper tile sizes: small at the edges (short pipeline fill/drain),
    `tt` rows/partition in the middle."""
    ramp = [r for r in (1, 1, 2) if r < tt]
    plan = list(ramp)
    mid = rows_per_part - 2 * sum(ramp)
    assert mid >= 0 and mid % tt == 0, (rows_per_part, tt)
    plan += [tt] * (mid // tt)
    plan += ramp[::-1]
    assert sum(plan) == rows_per_part
    return plan


def _build_program(
    rows=ROWS_PER_CORE,
    debug=False,
    tt=T,
    bufs=6,
    inplace=True,
    act_squares=0,
    taper=False,
):
    import concourse.bacc as bacc
    import concourse.mybir as mybir
    import concourse.tile as tile

    f32 = mybir.dt.float32
    nc = bacc.Bacc("TRN2", target_bir_lowering=False, debug=debug)
    x = nc.dram_tensor("x", [rows, D], f32, kind="ExternalInput").ap()
    out = nc.dram_tensor("out", [rows, D], f32, kind="ExternalOutput").ap()

    rows_per_part = rows // 128
    plan = _tile_plan(rows_per_part, tt) if taper else [tt] * (rows_per_part // tt)

    with tile.TileContext(nc) as tc:
        with (
            tc.tile_pool(name="xt", bufs=bufs) as xpool,
            tc.tile_pool(name="ot", bufs=bufs) as opool,
            tc.tile_pool(name="sc", bufs=3) as scpool,
            tc.tile_pool(name="b", bufs=bufs) as bpool,
        ):
            row0 = 0  # global row offset of this block
            for st in plan:
                nrows = 128 * st
                # contiguous row block; partition p holds st consecutive rows
                xv = x[row0 : row0 + nrows, :].rearrange(
                    "(p t) d -> p (t d)", p=128
                )
                ov = out[row0 : row0 + nrows, :].rearrange(
                    "(p t) d -> p (t d)", p=128
                )
                row0 += nrows
                xt = xpool.tile([128, st * D], f32, tag="xt")
                nc.sync.dma_start(out=xt[:, : st * D], in_=xv)
                ot = xt if inplace else opool.tile([128, st * D], f32, tag="ot")
                b = bpool.tile([128, st], f32, tag="b")
                sq = scpool.tile([128, D], f32, tag="sq")
                for t in range(st):
                    blk = xt[:, t * D : (t + 1) * D]
                    if t < act_squares:
                        nc.scalar.activation(
                            out=sq[:],
                            in_=blk,
                            func=mybir.ActivationFunctionType.Square,
                            accum_out=b[:, t : t + 1],
                        )
                    else:
                        nc.vector.scalar_tensor_tensor(
                            out=sq[:],
                            in0=blk,
                            scalar=1.0,
                            in1=blk,
                            op0=mybir.AluOpType.mult,
                            op1=mybir.AluOpType.mult,
                            accum_out=b[:, t : t + 1],
                        )
                # b = 1 + ||x_row||^2
                nc.vector.tensor_scalar_add(
                    out=b[:, :st], in0=b[:, :st], scalar1=1.0
                )
                for t in range(st):
                    blk = xt[:, t * D : (t + 1) * D]
                    # sqrt(-2*x + (1 + ||x_row||^2))
                    nc.scalar.activation(
                        out=ot[:, t * D : (t + 1) * D],
                        in_=blk,
                        func=mybir.ActivationFunctionType.Sqrt,
                        bias=b[:, t : t + 1],
                        scale=-2.0,
                    )
                # negate the whole super-tile in one op
                nc.vector.tensor_scalar_mul(
                    out=ot[:, : st * D], in0=ot[:, : st * D], scalar1=-1.0
                )
                nc.sync.dma_start(out=ov, in_=ot[:, : st * D])
    nc.finalize()
    return nc


def _build_program_raw(rows=ROWS_PER_CORE, debug=False, tt=T, nslots=4, out_q="sync"):
    """Hand-scheduled variant: explicit semaphores, no Tile framework.

    Per block s (slot = s % nslots), all in-place on xt[:, slot]:
      SP   : in-DMA(s)  [waits out-DMA(s-nslots) done]
             out-DMA(s) [waits negate(s) done]
      DVE  : stt x4 (row sums) -> badd (+1) -> negate [waits sqrt(s) done]
      ACT  : sqrt x4 [waits badd(s) done]
    """
    import concourse.bacc as bacc
    import concourse.mybir as mybir

    f32 = mybir.dt.float32
    nc = bacc.Bacc("TRN2", target_bir_lowering=False, debug=debug)
    x = nc.dram_tensor("x", [rows, D], f32, kind="ExternalInput").ap()
    out = nc.dram_tensor("out", [rows, D], f32, kind="ExternalOutput").ap()
    xv = x.rearrange("(s p t) d -> s p (t d)", p=128, t=tt)
    ov = out.rearrange("(s p t) d -> s p (t d)", p=128, t=tt)
    nsup = rows // (128 * tt)
    B = nslots

    xt = nc.alloc_sbuf_tensor("xt", [128, B, tt * D], f32).ap()
    bb = nc.alloc_sbuf_tensor("bb", [128, B, tt], f32).ap()
    sq = nc.alloc_sbuf_tensor("sq", [128, D], f32).ap()

    # per-block dve_sem watermarks, filled during DVE emission
    badd_done = {}
    neg_done = {}
    # DVE completion counter (every DVE op incs dve_sem by 1)
    dve_k = [0]

    from contextlib import ExitStack

    with ExitStack() as ctx:
        in_sems = [ctx.enter_context(nc.semaphore(name=f"in{i}")) for i in range(B)]
        out_sems = [ctx.enter_context(nc.semaphore(name=f"ou{i}")) for i in range(B)]
        dve_sem = ctx.enter_context(nc.semaphore(name="dve_sem"))
        act_sem = ctx.enter_context(nc.semaphore(name="act_sem"))
        block = ctx.enter_context(nc.Block())

        def _dve(v, thunk):
            """Chain a DVE op on dve_sem (detector-visible program order)."""
            if dve_k[0]:
                v.wait_ge(dve_sem, dve_k[0])
            thunk().then_inc(dve_sem, 1)
            dve_k[0] += 1

        def _negate(v, s):
            v.wait_ge(act_sem, tt * (s + 1))
            _dve(
                v,
                lambda: nc.vector.tensor_scalar_mul(
                    out=xt[:, s % B], in0=xt[:, s % B], scalar1=-1.0
                ),
            )
            neg_done[s] = dve_k[0]

        @block.vector
        def _(v):
            for s in range(nsup):
                slot = s % B
                v.wait_ge(in_sems[slot], 16 * (s // B + 1))
                for t in range(tt):
                    blk = xt[:, slot, t * D : (t + 1) * D]
                    _dve(
                        v,
                        lambda blk=blk, t=t: nc.vector.scalar_tensor_tensor(
                            out=sq[:],
                            in0=blk,
                            scalar=1.0,
                            in1=blk,
                            op0=mybir.AluOpType.mult,
                            op1=mybir.AluOpType.mult,
                            accum_out=bb[:, slot, t : t + 1],
                        ),
                    )
                _dve(
                    v,
                    lambda slot=slot: nc.vector.tensor_scalar_add(
                        out=bb[:, slot], in0=bb[:, slot], scalar1=1.0
                    ),
                )
                badd_done[s] = dve_k[0]
                # negate lags one block so DVE never stalls behind ACT
                if s >= 1:
                    _negate(v, s - 1)
            _negate(v, nsup - 1)

        @block.scalar
        def _(a):
            j = 0
            for s in range(nsup):
                slot = s % B
                a.wait_ge(dve_sem, badd_done[s])
                for t in range(tt):
                    if j:
                        a.wait_ge(act_sem, j)
                    blk = xt[:, slot, t * D : (t + 1) * D]
                    nc.scalar.activation(
                        out=blk,
                        in_=blk,
                        func=mybir.ActivationFunctionType.Sqrt,
                        bias=bb[:, slot, t : t + 1],
                        scale=-2.0,
                    ).then_inc(act_sem, 1)
                    j += 1

        if out_q == "sync":

            @block.sync
            def _(sync):
                for s in range(nsup):
                    slot = s % B
                    if s >= B:
                        sync.wait_ge(out_sems[slot], 16 * (s // B))
                    sync.dma_start(out=xt[:, slot], in_=xv[s]).then_inc(
                        in_sems[slot], 16
                    )
                    if s >= B - 1:
                        # interleave: emit out-DMA for the oldest ready block
                        so = s - B + 1
                        sync.wait_ge(dve_sem, neg_done[so])
                        sync.dma_start(out=ov[so], in_=xt[:, so % B]).then_inc(
                            out_sems[so % B], 16
                        )
                for so in range(max(0, nsup - B + 1), nsup):
                    sync.wait_ge(dve_sem, neg_done[so])
                    sync.dma_start(out=ov[so], in_=xt[:, so % B]).then_inc(
                        out_sems[so % B], 16
                    )
        else:
            # in-DMAs on the SP HWDGE ring; out-DMAs on the gpsimd SWDGE
            # queue — two independent descriptor paths, no head-of-line
            @block.sync
            def _(sync):
                for s in range(nsup):
                    slot = s % B
                    if s >= B:
                        sync.wait_ge(out_sems[slot], 16 * (s // B))
                    sync.dma_start(out=xt[:, slot], in_=xv[s]).then_inc(
                        in_sems[slot], 16
                    )

            @block.gpsimd
            def _(g):
                for so in range(nsup):
                    g.wait_ge(dve_sem, neg_done[so])
                    nc.gpsimd.dma_start(out=ov[so], in_=xt[:, so % B]).then_inc(
                        out_sems[so % B], 16
                    )

    nc.finalize()
    return nc


def _run_device(x: np.ndarray, trace: bool = False):
    from concourse import bass_utils

    if "nc" not in _cache:
        _cache["nc"] = _build_program_raw(nslots=6)
    nc = _cache["nc"]
    shards = [
        np.ascontiguousarray(x[i * ROWS_PER_CORE : (i + 1) * ROWS_PER_CORE])
        for i in range(N_CORES)
    ]
    res = bass_utils.run_bass_kernel_spmd(
        nc,
        [{"x": s} for s in shards],
        core_ids=list(range(N_CORES)),
        trace=trace,
    )
    out = np.concatenate([r["out"] for r in res.results], axis=0)
    return out, res


def _fallback(x: np.ndarray, prototypes: np.ndarray) -> np.ndarray:
    x = x.astype(np.float32, copy=False)
    p = prototypes.astype(np.float32, copy=False)
    x_sq = np.sum(x * x, axis=1, keepdims=True)
    p_sq = np.sum(p * p, axis=1)
    cross = x @ p.T
    d2 = np.maximum(x_sq + p_sq[None, :] - 2.0 * cross, 0.0)
    return (-np.sqrt(d2)).astype(np.float32)


def _is_identity(p: np.ndarray) -> bool:
    if p.shape != (D, D):
        return False
    if "eye" not in _cache:
        _cache["eye"] = np.eye(D, dtype=np.float32)
    return np.array_equal(np.asarray(p, dtype=np.float32), _cache["eye"])


def kernel(x: np.ndarray, prototypes: np.ndarray) -> np.ndarray:
    x = np.asarray(x)
    prototypes = np.asarray(prototypes)
    if (
        x.shape == (N_ROWS, D)
        and x.dtype == np.float32
        and _is_identity(prototypes)
    ):
        out, _ = _run_device(x)
        return out
    return _fallback(x, prototypes)


# revision 25
# speedup vs baseline: 6.8252x; 1.0051x over previous
"""Trainium2 Bass kernel for nn_RadialPredictionLayer (retrieval_knn).

Computes out[n, c] = -sqrt(max(||x_n||^2 + ||p_c||^2 - 2 * x_n . p_c, 0))
for x [32768, 1024] fp32 and prototypes [1024, 1024] fp32.

The layer's prototypes are a fixed (non-trainable) identity matrix, so the
device kernel specializes on that constant (verified at runtime):
    cross = x @ I^T = x,  ||p_c||^2 = 1
    out[n, c] = -sqrt(1 + ||x_n||^2 - 2 * x[n, c])
which is a pure memory-bound elementwise + row-reduction kernel (no GEMM).
Sharding: data-parallel on the batch axis across 8 NeuronCores; each core
processes a [4096, 1024] row block. If prototypes is ever not the identity,
a host-side exact fallback implements the general formula.
"""

import numpy as np

N_CORES = 8
N_ROWS = 32768
D = 1024
ROWS_PER_CORE = N_ROWS // N_CORES  # 4096
T = 4  # rows per partition per super-tile
SUP = ROWS_PER_CORE // (128 * T)  # super-tiles per core

_cache = {}


def _tile_plan(rows_per_part, tt):
    """Taper tile sizes: small at the edges (short pipeline fill/drain),
    `tt` rows/partition in the middle."""
    ramp = [r for r in (1, 1, 2) if r < tt]
    plan = list(ramp)
    mid = rows_per_part - 2 * sum(ramp)
    assert mid >= 0 and mid % tt == 0, (rows_per_part, tt)
    plan += [tt] * (mid // tt)
    plan += ramp[::-1]
    assert sum(plan) == rows_per_part
    return plan


def _build_program(
    rows=ROWS_PER_CORE,
    debug=False,
    tt=T,
    bufs=6,
    inplace=True,
    act_squares=0,
    taper=False,
):
    import concourse.bacc as bacc
    import concourse.mybir as mybir
    import concourse.tile as tile

    f32 = mybir.dt.float32
    nc = bacc.Bacc("TRN2", target_bir_lowering=False, debug=debug)
    x = nc.dram_tensor("x", [rows, D], f32, kind="ExternalInput").ap()
    out = nc.dram_tensor("out", [rows, D], f32, kind="ExternalOutput").ap()

    rows_per_part = rows // 128
    plan = _tile_plan(rows_per_part, tt) if taper else [tt] * (rows_per_part // tt)

    with tile.TileContext(nc) as tc:
        with (
            tc.tile_pool(name="xt", bufs=bufs) as xpool,
            tc.tile_pool(name="ot", bufs=bufs) as opool,
            tc.tile_pool(name="sc", bufs=3) as scpool,
            tc.tile_pool(name="b", bufs=bufs) as bpool,
        ):
            row0 = 0  # global row offset of this block
            for st in plan:
                nrows = 128 * st
                # contiguous row block; partition p holds st consecutive rows
                xv = x[row0 : row0 + nrows, :].rearrange(
                    "(p t) d -> p (t d)", p=128
                )
                ov = out[row0 : row0 + nrows, :].rearrange(
                    "(p t) d -> p (t d)", p=128
                )
                row0 += nrows
                xt = xpool.tile([128, st * D], f32, tag="xt")
                nc.sync.dma_start(out=xt[:, : st * D], in_=xv)
                ot = xt if inplace else opool.tile([128, st * D], f32, tag="ot")
                b = bpool.tile([128, st], f32, tag="b")
                sq = scpool.tile([128, D], f32, tag="sq")
                for t in range(st):
                    blk = xt[:, t * D : (t + 1) * D]
                    if t < act_squares:
                        nc.scalar.activation(
                            out=sq[:],
                            in_=blk,
                            func=mybir.ActivationFunctionType.Square,
                            accum_out=b[:, t : t + 1],
                        )
                    else:
                        nc.vector.scalar_tensor_tensor(
                            out=sq[:],
                            in0=blk,
                            scalar=1.0,
                            in1=blk,
                            op0=mybir.AluOpType.mult,
                            op1=mybir.AluOpType.mult,
                            accum_out=b[:, t : t + 1],
                        )
                # b = 1 + ||x_row||^2
                nc.vector.tensor_scalar_add(
                    out=b[:, :st], in0=b[:, :st], scalar1=1.0
                )
                for t in range(st):
                    blk = xt[:, t * D : (t + 1) * D]
                    # sqrt(-2*x + (1 + ||x_row||^2))
                    nc.scalar.activation(
                        out=ot[:, t * D : (t + 1) * D],
                        in_=blk,
                        func=mybir.ActivationFunctionType.Sqrt,
                        bias=b[:, t : t + 1],
                        scale=-2.0,
                    )
                # negate the whole super-tile in one op
                nc.vector.tensor_scalar_mul(
                    out=ot[:, : st * D], in0=ot[:, : st * D], scalar1=-1.0
                )
                nc.sync.dma_start(out=ov, in_=ot[:, : st * D])
    nc.finalize()
    return nc


def _build_program_raw(
    rows=ROWS_PER_CORE,
    debug=False,
    tt=T,
    nslots=4,
    out_q="sync",
    end_taper=(),
    asserts=True,
):
    """Hand-scheduled variant: explicit semaphores, no Tile framework.

    Per block s (slot = s % nslots), all in-place on xt[:, slot]:
      SP   : in-DMA(s)  [waits out-DMA(s-nslots) done]
             out-DMA(s) [waits negate(s) done]
      DVE  : stt per row (row sums) -> badd (+1) -> negate [waits sqrt(s) done]
      ACT  : sqrt per row [waits badd(s) done]
    `end_taper`: optional final block sizes (rows/partition), e.g. (2, 1, 1),
    shortening the last compute chains so the DMA drains without gaps.
    """
    import concourse.bacc as bacc
    import concourse.mybir as mybir

    f32 = mybir.dt.float32
    nc = bacc.Bacc(
        "TRN2", target_bir_lowering=False, debug=debug, enable_asserts=asserts
    )
    x = nc.dram_tensor("x", [rows, D], f32, kind="ExternalInput").ap()
    out = nc.dram_tensor("out", [rows, D], f32, kind="ExternalOutput").ap()

    rpp = rows // 128
    mid = rpp - sum(end_taper)
    assert mid >= 0 and mid % tt == 0, (rpp, tt, end_taper)
    plan = [tt] * (mid // tt) + list(end_taper)
    nsup = len(plan)
    B = nslots

    # per-block contiguous-row views: block s holds 128*plan[s] rows
    xvs, ovs = [], []
    row0 = 0
    for st in plan:
        nrows = 128 * st
        xvs.append(x[row0 : row0 + nrows, :].rearrange("(p t) d -> p (t d)", p=128))
        ovs.append(out[row0 : row0 + nrows, :].rearrange("(p t) d -> p (t d)", p=128))
        row0 += nrows
    assert row0 == rows

    xt = nc.alloc_sbuf_tensor("xt", [128, B, tt * D], f32).ap()
    bb = nc.alloc_sbuf_tensor("bb", [128, B, tt], f32).ap()
    sq = nc.alloc_sbuf_tensor("sq", [128, D], f32).ap()

    # per-block dve_sem / act_sem watermarks, filled during DVE/ACT emission
    badd_done = {}
    neg_done = {}
    sq_done = [0]
    for s, st in enumerate(plan):
        sq_done.append(sq_done[-1] + st)
    # DVE completion counter (every DVE op incs dve_sem by 1)
    dve_k = [0]

    from contextlib import ExitStack

    with ExitStack() as ctx:
        in_sems = [ctx.enter_context(nc.semaphore(name=f"in{i}")) for i in range(B)]
        out_sems = [ctx.enter_context(nc.semaphore(name=f"ou{i}")) for i in range(B)]
        dve_sem = ctx.enter_context(nc.semaphore(name="dve_sem"))
        act_sem = ctx.enter_context(nc.semaphore(name="act_sem"))

        # Semaphores are NOT cleared by allocation, and with
        # target_bir_lowering=False Bass emits no clear preamble — a prior
        # NEFF on this core may have left nonzero values that would let our
        # absolute-value waits pass spuriously. Clear them, then barrier.
        from concourse.bass import compact_to_ranges

        sem_nums = sorted(
            s.num for s in (*in_sems, *out_sems, dve_sem, act_sem)
        )
        for sem_range in compact_to_ranges(sem_nums):
            nc.gpsimd.dma_reset(sem_range)
            nc.gpsimd.sem_clear(sem_range)
        nc.all_engine_barrier()

        block = ctx.enter_context(nc.Block())

        def _dve(v, thunk):
            """Chain a DVE op on dve_sem (detector-visible program order)."""
            if dve_k[0]:
                v.wait_ge(dve_sem, dve_k[0])
            thunk().then_inc(dve_sem, 1)
            dve_k[0] += 1

        def _negate(v, s):
            st = plan[s]
            v.wait_ge(act_sem, sq_done[s + 1])
            _dve(
                v,
                lambda: nc.vector.tensor_scalar_mul(
                    out=xt[:, s % B, : st * D],
                    in0=xt[:, s % B, : st * D],
                    scalar1=-1.0,
                ),
            )
            neg_done[s] = dve_k[0]

        @block.vector
        def _(v):
            for s, st in enumerate(plan):
                slot = s % B
                v.wait_ge(in_sems[slot], 16 * (s // B + 1))
                for t in range(st):
                    blk = xt[:, slot, t * D : (t + 1) * D]
                    _dve(
                        v,
                        lambda blk=blk, slot=slot, t=t: nc.vector.scalar_tensor_tensor(
                            out=sq[:],
                            in0=blk,
                            scalar=1.0,
                            in1=blk,
                            op0=mybir.AluOpType.mult,
                            op1=mybir.AluOpType.mult,
                            accum_out=bb[:, slot, t : t + 1],
                        ),
                    )
                _dve(
                    v,
                    lambda slot=slot, st=st: nc.vector.tensor_scalar_add(
                        out=bb[:, slot, :st], in0=bb[:, slot, :st], scalar1=1.0
                    ),
                )
                badd_done[s] = dve_k[0]
                # negate lags one block so DVE never stalls behind ACT
                if s >= 1:
                    _negate(v, s - 1)
            _negate(v, nsup - 1)

        @block.scalar
        def _(a):
            j = 0
            for s, st in enumerate(plan):
                slot = s % B
                a.wait_ge(dve_sem, badd_done[s])
                for t in range(st):
                    if j:
                        a.wait_ge(act_sem, j)
                    blk = xt[:, slot, t * D : (t + 1) * D]
                    nc.scalar.activation(
                        out=blk,
                        in_=blk,
                        func=mybir.ActivationFunctionType.Sqrt,
                        bias=bb[:, slot, t : t + 1],
                        scale=-2.0,
                    ).then_inc(act_sem, 1)
                    j += 1

        def _emit_out(eng, dma_engine, so):
            eng.wait_ge(dve_sem, neg_done[so])
            dma_engine.dma_start(
                out=ovs[so], in_=xt[:, so % B, : plan[so] * D]
            ).then_inc(out_sems[so % B], 16)

        if out_q == "sync":

            @block.sync
            def _(sync):
                for s in range(nsup):
                    slot = s % B
                    if s >= B:
                        sync.wait_ge(out_sems[slot], 16 * (s // B))
                    sync.dma_start(
                        out=xt[:, slot, : plan[s] * D], in_=xvs[s]
                    ).then_inc(in_sems[slot], 16)
                    if s >= B - 1:
                        # interleave: emit out-DMA for the oldest ready block
                        _emit_out(sync, sync, s - B + 1)
                for so in range(max(0, nsup - B + 1), nsup):
                    _emit_out(sync, sync, so)
        else:
            # in-DMAs on the SP HWDGE ring; out-DMAs on the gpsimd SWDGE
            # queue — two independent descriptor paths, no head-of-line
            @block.sync
            def _(sync):
                for s in range(nsup):
                    slot = s % B
                    if s >= B:
                        sync.wait_ge(out_sems[slot], 16 * (s // B))
                    sync.dma_start(
                        out=xt[:, slot, : plan[s] * D], in_=xvs[s]
                    ).then_inc(in_sems[slot], 16)

            @block.gpsimd
            def _(g):
                for so in range(nsup):
                    _emit_out(g, nc.gpsimd, so)

    nc.finalize()
    return nc


def _run_device(x: np.ndarray, trace: bool = False):
    from concourse import bass_utils

    if "nc" not in _cache:
        _cache["nc"] = _build_program_raw(nslots=8, end_taper=(2, 1, 1))
    nc = _cache["nc"]
    shards = [
        np.ascontiguousarray(x[i * ROWS_PER_CORE : (i + 1) * ROWS_PER_CORE])
        for i in range(N_CORES)
    ]
    res = bass_utils.run_bass_kernel_spmd(
        nc,
        [{"x": s} for s in shards],
        core_ids=list(range(N_CORES)),
        trace=trace,
    )
    out = np.concatenate([r["out"] for r in res.results], axis=0)
    return out, res


def _fallback(x: np.ndarray, prototypes: np.ndarray) -> np.ndarray:
    x = x.astype(np.float32, copy=False)
    p = prototypes.astype(np.float32, copy=False)
    x_sq = np.sum(x * x, axis=1, keepdims=True)
    p_sq = np.sum(p * p, axis=1)
    cross = x @ p.T
    d2 = np.maximum(x_sq + p_sq[None, :] - 2.0 * cross, 0.0)
    return (-np.sqrt(d2)).astype(np.float32)


def _is_identity(p: np.ndarray) -> bool:
    if p.shape != (D, D):
        return False
    if "eye" not in _cache:
        _cache["eye"] = np.eye(D, dtype=np.float32)
    return np.array_equal(np.asarray(p, dtype=np.float32), _cache["eye"])


def kernel(x: np.ndarray, prototypes: np.ndarray) -> np.ndarray:
    x = np.asarray(x)
    prototypes = np.asarray(prototypes)
    if (
        x.shape == (N_ROWS, D)
        and x.dtype == np.float32
        and _is_identity(prototypes)
    ):
        out, _ = _run_device(x)
        return out
    return _fallback(x, prototypes)
